# revision 10
# baseline (speedup 1.0000x reference)
"""Trainium2 Bass kernel for nn_Attention_13700945674736 (sparse local-window attention).

Strategy (8 NeuronCores, data-parallel over batch, 4 samples/core):
  - Permute the sequence axis s = 64*i + j  ->  s' = 16*j + i (image transpose).
    The 7x11 local window becomes a 1-D band |ds'| <= 83, so each 128-query
    tile only attends to 3 aligned 128-key chunks instead of 1024.
  - Exact per-chunk band is [128c-80, 128c+208): pairs at distance 81..83
    need |dj|=5 AND |di|>=1 simultaneously, which the 2-D mask forbids for
    the first/last 3 keys of a chunk, so halfwidth 80 suffices on the low
    side (and symmetrically 79+128 on the high side; 208 = 128+80).
  - Heads are padded to 64-partition slots (host-padded weights) so every
    engine access pattern starts at a 32-aligned partition.
  - All matmul operands are bf16; PSUM accumulation and softmax reductions
    stay fp32.
  - attnT[k, q] band tiles; exp on ScalarE (3 chunk-groups per head -> few,
    large activations); binary window mask applied multiplicatively, split
    between DVE and GPSIMD(Pool) for engine balance; @V uses
    lhsT=[V|0|ones|0] so softmax denominators land at partitions 64:112 of
    the same PSUM tile; the projection bias is folded in via a constant-1
    row of aoT.
  - po is a single persistent 2-bank PSUM tile; q-tile t of every head uses
    column (t%2)*512+(t//2)*128 so concurrently-open accumulation groups
    alternate banks.  Closed tiles keep their data across same-bank
    start=True (only has_written bits are cleared), so the per-head divides
    run in 2 halves while the next head's @V already accumulates.
  - PSUM budget: pat 2x[128,1024] (4 banks) + proj 2x[128,512] (2 banks)
    + po [128,1024] (2 banks) = 8 banks exactly.
  - Evacuation copies (Q/K/V/out PSUM->SBUF) are split between ScalarE and
    DVE by static schedule tables; output stores go through the SP queue so
    DMA setup does not stall the Activation sequencer.
"""

import sys

sys.path.insert(0, "/opt/trn_rl_repo")

import numpy as np

import concourse.bass as bass
from concourse import bacc
import concourse.mybir as mybir
import concourse.tile as tile
from concourse.bass_utils import run_bass_kernel_spmd

# ---------------------------------------------------------------- constants
B, S, C = 32, 1024, 384
H, D = 8, 48
HI, WI = 16, 64
N_CORES = 8
BL = B // N_CORES  # samples per core
SCALE = float(D) ** -0.5
F32 = mybir.dt.float32
BF16 = mybir.dt.bfloat16
PD = BF16  # precision of expT / m01 / vv

# s' = 16*j + i  <->  s = 64*i + j ;  PERM[s'] = s
_sp = np.arange(S)
PERM = (_sp % HI) * WI + (_sp // HI)

NQT = S // 128  # 8 query tiles (and key chunks)
WPADQ = 64 * H  # padded Q (and K) section width: 512
WQW = 2 * WPADQ + C  # 1408

# exact per-chunk bands: key-chunk c attends to queries [QLO[c], QHI[c])
QLO = [max(0, 128 * c - 80) for c in range(NQT)]
QHI = [min(S, 128 * c + 208) for c in range(NQT)]
WC = [QHI[c] - QLO[c] for c in range(NQT)]
OFFC = list(np.cumsum([0] + WC[:-1]))
BAND_W = sum(WC)  # 2144

# chunk groups per PSUM pat tile (each group width <= 1024 f32 = 2 banks)
CH_GROUPS = [(0, 1, 2), (3, 4, 5), (6, 7)]
GRP_BASE = [OFFC[g[0]] for g in CH_GROUPS]
GRP_W = [OFFC[g[-1]] + WC[g[-1]] - OFFC[g[0]] for g in CH_GROUPS]
assert max(GRP_W) <= 1024

# po column slot for q-tile t (identical for every head; open groups t,t+1
# always land in different PSUM banks)
PO_COL = [(t % 2) * 512 + (t // 2) * 128 for t in range(NQT)]

# ------------------------------------------------- engine schedule tables
# 'a' = ScalarE(Act) copy, 'd' = DVE tensor_copy
QK_EVAC = "aadaaadaaadaaada"  # 16 per sample (Q then K, pair-major)
V_EVAC = "aadaaada"  # 8 per sample
OUT_EVAC = "aadaaada"  # 8 per sample
# 'd' = DVE, 'p' = Pool(GPSIMD) for the 3 mask multiplies of each head.
# g1 (chunks 3-5) sits mid-head where its latency hides; g0/g2 gate the
# @V start/tail chains, so they stay on the fast DVE.
MASK_ENG = ["dpd"] * H
# divide granularity: q-tile ranges; later ranges close later, and finer
# tail pieces release the po columns the next head's early @V chunks need.
DIV_RANGES = [(0, 4), (4, 6), (6, 8)]

# ---------------------------------------------------------------- bass program
_CACHE = {}


def _build():
    if "nc" in _CACHE:
        return _CACHE["nc"]

    nc = bacc.Bacc(None, target_bir_lowering=False)
    xT_d = nc.declare_dram_parameter("xT", [BL, C, S], BF16, isOutput=False)
    wq_d = nc.declare_dram_parameter("wq_pad", [C, WQW], BF16, isOutput=False)
    wp_d = nc.declare_dram_parameter("wp_pad", [4, 128, C], BF16, isOutput=False)
    ones_d = nc.declare_dram_parameter("ones_row", [1, S], BF16, isOutput=False)
    m_d = nc.declare_dram_parameter("m01", [128, BAND_W], PD, isOutput=False)
    out_d = nc.declare_dram_parameter("out", [BL, S, C], F32, isOutput=True)

    with tile.TileContext(nc) as tc:
        with (
            tc.tile_pool(name="singles", bufs=1) as singles,
            tc.tile_pool(name="xt_pool", bufs=3) as xt_pool,
            tc.tile_pool(name="out_pool", bufs=4) as out_pool,
            tc.tile_pool(name="ps_proj", bufs=2, space="PSUM") as ps_proj,
            tc.tile_pool(name="ps_pat", bufs=2, space="PSUM") as ps_pat,
            tc.tile_pool(name="ps_po", bufs=1, space="PSUM") as ps_po,
        ):
            # ---- constants.  Q/K/V weight sections load as separate DMAs so
            # the first projection matmuls only wait for their own section.
            w_sb = singles.tile([128, 3, WQW], BF16)
            wq_v = wq_d.rearrange("(c p) w -> p c w", p=128)
            nc.scalar.dma_start(w_sb[:, :, 0:WPADQ], wq_v[:, :, 0:WPADQ])
            nc.scalar.dma_start(
                w_sb[:, :, WPADQ : 2 * WPADQ], wq_v[:, :, WPADQ : 2 * WPADQ]
            )
            nc.scalar.dma_start(w_sb[:, :, 2 * WPADQ :], wq_v[:, :, 2 * WPADQ :])
            wp_sb = singles.tile([128, 4, C], BF16)
            nc.gpsimd.dma_start(wp_sb[:, :, :], wp_d.rearrange("f p c -> p f c"))
            m_sb = singles.tile([128, BAND_W], PD)
            nc.gpsimd.dma_start(m_sb, m_d[:, :])

            # persistent attention-output PSUM tile (2 banks)
            po = ps_po.tile([128, S], F32, tag="po")

            # ---- per-sample tiles, double-buffered for cross-sample overlap
            qTs, kTs, vvs, aoTs, expTs, dens = [], [], [], [], [], []
            for i in range(2):
                qTs.append(singles.tile([128, 4, S], BF16, name=f"qT{i}"))
                kTs.append(singles.tile([128, 4, S], BF16, name=f"kT{i}"))
                vvs.append(singles.tile([128, NQT, H, 128], PD, name=f"vv{i}"))
                aoTs.append(singles.tile([128, 4, S], BF16, name=f"aoT{i}"))
                expTs.append(singles.tile([128, BAND_W], PD, name=f"expT{i}"))
                dens.append(singles.tile([48, 2, 512], F32, name=f"den{i}"))
            # setup memsets run on DVE (idle during the startup DMA waits)
            for vv in vvs:
                nc.vector.memset(vv[:, :, :, D : D + 16], 0.0)
                nc.vector.memset(vv[:, :, :, D + 16 : 112], 1.0)
                nc.vector.memset(vv[:, :, :, 112:128], 0.0)
            for aoT in aoTs:
                # zero dead rows (48:64, 112:128); starts must be 32-aligned so
                # cover 32:64 / 96:128 — live rows are rewritten by the divides.
                nc.vector.memset(aoT[32:64, :, :], 0.0)
                nc.vector.memset(aoT[96:128, :, :], 0.0)
                # constant-1 row: proj picks up b_proj from wp_pad[0][48]
                nc.gpsimd.dma_start(aoT[48:49, 0, :], ones_d[:, :])

            def evac(engine, dst, src):
                if engine == "a":
                    nc.scalar.copy(dst, src)
                else:
                    nc.vector.tensor_copy(dst, src)

            def load_xt(b):
                xt = xt_pool.tile([128, 3, S], BF16)
                nc.sync.dma_start(
                    xt[:, :, :], xT_d[b].rearrange("(c p) s -> p c s", p=128)
                )
                return xt

            def proj(b, xt):
                """QKV projection for sample b into qT/kT/vv set b%2."""
                qT, kT, vv = qTs[b % 2], kTs[b % 2], vvs[b % 2]
                ei = iter(QK_EVAC)
                for qk in range(2):
                    dst = qT if qk == 0 else kT
                    for pair in range(4):
                        ncol = qk * WPADQ + pair * 128
                        for half in range(2):
                            ps = ps_proj.tile([128, 512], F32, tag="mm")
                            for ci in range(3):
                                nc.tensor.matmul(
                                    ps[:, :],
                                    w_sb[:, ci, ncol : ncol + 128],
                                    xt[:, ci, half * 512 : (half + 1) * 512],
                                    start=(ci == 0),
                                    stop=(ci == 2),
                                )
                            seg = dst[:, pair, half * 512 : (half + 1) * 512]
                            evac(next(ei), seg, ps[:, :])
                vi = iter(V_EVAC)
                for st in range(NQT):
                    psv = ps_proj.tile([128, 512], F32, tag="mm")
                    for ci in range(3):
                        nc.tensor.matmul(
                            psv[:, 0:C],
                            xt[:, ci, st * 128 : (st + 1) * 128],
                            w_sb[:, ci, 2 * WPADQ : 2 * WPADQ + C],
                            start=(ci == 0),
                            stop=(ci == 2),
                        )
                    evac(
                        next(vi),
                        vv[:, st, :, 0:D],
                        psv[:, 0:C].rearrange("p (h d) -> p h d", h=H),
                    )

            def attention(b):
                qT, kT, vv, aoT = qTs[b % 2], kTs[b % 2], vvs[b % 2], aoTs[b % 2]
                for h in range(H):
                    pair, sub = divmod(h, 2)
                    p0 = sub * 64
                    expT = expTs[h % 2]
                    den_sb = dens[h % 2]
                    # ---- QK^T band, exp, mask (3 chunk-groups)
                    for gi, grp in enumerate(CH_GROUPS):
                        gbase, gw = GRP_BASE[gi], GRP_W[gi]
                        pat = ps_pat.tile([128, 1024], F32, tag="attn")
                        for c in grp:
                            lo = OFFC[c] - gbase
                            hi = lo + WC[c]
                            a = lo
                            while a < hi:
                                b2 = min(hi, (a // 512 + 1) * 512)
                                nc.tensor.matmul(
                                    pat[:, a:b2],
                                    kT[p0 : p0 + D, pair, c * 128 : (c + 1) * 128],
                                    qT[
                                        p0 : p0 + D,
                                        pair,
                                        QLO[c] + (a - lo) : QLO[c] + (b2 - lo),
                                    ],
                                    start=True,
                                    stop=True,
                                )
                                a = b2
                        nc.scalar.activation(
                            expT[:, gbase : gbase + gw],
                            pat[:, 0:gw],
                            mybir.ActivationFunctionType.Exp,
                            scale=SCALE,
                        )
                        eng = MASK_ENG[h][gi]
                        tt = (
                            nc.vector.tensor_tensor
                            if eng == "d"
                            else nc.gpsimd.tensor_tensor
                        )
                        tt(
                            expT[:, gbase : gbase + gw],
                            expT[:, gbase : gbase + gw],
                            m_sb[:, gbase : gbase + gw],
                            mybir.AluOpType.mult,
                        )
                    # ---- @V with ones rows at 64:112 -> denominators
                    for c in range(NQT):
                        lhsT = vv[:, c, h, :]
                        for t in range(max(c - 1, 0), min(c + 2, NQT)):
                            pc = PO_COL[t]
                            qs = max(128 * t, QLO[c])
                            qe = min(128 * t + 128, QHI[c])
                            nc.tensor.matmul(
                                po[:, pc + (qs - 128 * t) : pc + (qe - 128 * t)],
                                lhsT,
                                expT[:, OFFC[c] + (qs - QLO[c]) : OFFC[c] + (qe - QLO[c])],
                                start=(c == max(t - 1, 0)),
                                stop=(c == min(t + 1, NQT - 1)),
                            )
                    # ---- normalize in q-tile ranges: later ranges close later,
                    # finer tail pieces release po columns for the next head.
                    # q-order view of po: q-tile t = 2*thi + tlo lives at
                    # col tlo*512 + thi*128, so iterate (thi, tlo, u).
                    po_q = po[:, :].rearrange(
                        "p (tlo thi u) -> p thi tlo u", tlo=2, thi=4, u=128
                    )
                    den_flat = den_sb[:, :, :].rearrange("p a b -> p (a b)")
                    for ta, tb in DIV_RANGES:
                        nt = (tb - ta) // 2
                        den_v = den_flat[:, ta * 128 : tb * 128].rearrange(
                            "p (thi tlo u) -> p thi tlo u", thi=nt, tlo=2, u=128
                        )
                        ao_v = aoT[
                            p0 : p0 + D, pair, ta * 128 : tb * 128
                        ].rearrange("p (thi tlo u) -> p thi tlo u", thi=nt, tlo=2, u=128)
                        nc.vector.reciprocal(
                            den_v, po_q[64 : 64 + D, ta // 2 : tb // 2]
                        )
                        nc.vector.tensor_tensor(
                            ao_v,
                            po_q[0:D, ta // 2 : tb // 2],
                            den_v,
                            mybir.AluOpType.mult,
                        )

            def out_proj(b):
                aoT = aoTs[b % 2]
                oi = iter(OUT_EVAC)
                for sp in range(NQT // 2):  # store pairs of q-tiles
                    ot = out_pool.tile([128, 2, C], F32)
                    for k in range(2):
                        st = 2 * sp + k
                        psp = ps_proj.tile([128, 512], F32, tag="mm")
                        for p in range(4):
                            nc.tensor.matmul(
                                psp[:, 0:C],
                                aoT[:, p, st * 128 : (st + 1) * 128],
                                wp_sb[:, p, :],
                                start=(p == 0),
                                stop=(p == 3),
                            )
                        evac(next(oi), ot[:, k, :], psp[:, 0:C])
                    nc.sync.dma_start(
                        out_d[b, 2 * sp * 128 : (2 * sp + 2) * 128, :].rearrange(
                            "(k p) c -> p k c", k=2
                        ),
                        ot[:, :, :],
                    )

            # ---------------- main pipeline
            xts = {0: load_xt(0)}
            proj(0, xts[0])
            for b in range(BL):
                if b + 1 < BL:
                    xts[b + 1] = load_xt(b + 1)  # prefetch during attention(b)
                attention(b)
                if b + 1 < BL:
                    proj(b + 1, xts[b + 1])
                out_proj(b)

    nc.finalize()
    _CACHE["nc"] = nc
    return nc


# ---------------------------------------------------------------- host wrapper
def _np_bf16(a):
    import ml_dtypes

    return np.asarray(a, dtype=ml_dtypes.bfloat16)


def _build_m01(mask):
    """[128, BAND_W] banded 0/1 mask in exact-band layout (rows = key within
    chunk c, cols = q in [QLO[c], QHI[c]))."""
    mp = np.asarray(mask)[np.ix_(PERM, PERM)]
    good = np.isfinite(mp) & (mp == 0.0)
    m01 = np.zeros((128, BAND_W), np.float32)
    covered = 0
    for c in range(NQT):
        blk = good[QLO[c] : QHI[c], c * 128 : (c + 1) * 128]  # [q, k]
        m01[:, OFFC[c] : OFFC[c] + WC[c]] = blk.T.astype(np.float32)
        covered += int(blk.sum())
    assert covered == int(good.sum()), "mask not covered by band layout"
    return m01


def _pad_wqkv(w_qkv):
    """[384, 1152] -> [384, 1408]: Q/K head h at cols h*64..h*64+48 (zero pad),
    V kept natural at cols 1024:1408."""
    out = np.zeros((C, WQW), np.float32)
    for sec in range(2):  # Q, K
        for h in range(H):
            out[:, sec * WPADQ + h * 64 : sec * WPADQ + h * 64 + D] = w_qkv[
                :, sec * C + h * D : sec * C + (h + 1) * D
            ]
    out[:, 2 * WPADQ :] = w_qkv[:, 2 * C :]
    return out


def _pad_wproj(w_proj, b_proj):
    """[384, 384] -> [4, 128, 384]: pair p rows 0:48 = head 2p, 64:112 = head 2p+1.
    Row 48 of pair 0 carries b_proj (matched by the constant-1 row in aoT)."""
    out = np.zeros((4, 128, C), np.float32)
    for p in range(4):
        out[p, 0:D] = w_proj[(2 * p) * D : (2 * p + 1) * D]
        out[p, 64 : 64 + D] = w_proj[(2 * p + 1) * D : (2 * p + 2) * D]
    out[0, D] = b_proj
    return out


def kernel(x, w_qkv, w_proj, b_proj, mask):
    x = np.asarray(x, np.float32)
    w_qkv = np.asarray(w_qkv, np.float32)
    w_proj = np.asarray(w_proj, np.float32)
    b_proj = np.asarray(b_proj, np.float32)

    nc = _build()

    xT = _np_bf16(np.ascontiguousarray(x[:, PERM, :].transpose(0, 2, 1)))  # [B, C, S']
    wq_pad = _np_bf16(_pad_wqkv(w_qkv))
    wp_pad = _np_bf16(_pad_wproj(w_proj, b_proj))
    ones_row = _np_bf16(np.ones((1, S), np.float32))
    m01 = _build_m01(mask)
    if PD == BF16:
        m01 = _np_bf16(m01)

    in_maps = [
        {
            "xT": xT[c * BL : (c + 1) * BL],
            "wq_pad": wq_pad,
            "wp_pad": wp_pad,
            "ones_row": ones_row,
            "m01": m01,
        }
        for c in range(N_CORES)
    ]
    res = run_bass_kernel_spmd(nc, in_maps, list(range(N_CORES)))
    out_p = np.concatenate([res.results[c]["out"] for c in range(N_CORES)], axis=0)
    out = np.empty_like(out_p)
    out[:, PERM, :] = out_p
    return out


# revision 12
# speedup vs baseline: 1.0622x; 1.0622x over previous
"""Trainium2 Bass kernel for nn_Attention_13700945674736 (sparse local-window attention).

Strategy (8 NeuronCores, data-parallel over batch, 4 samples/core):
  - Permute the sequence axis s = 64*i + j  ->  s' = 16*j + i (image transpose).
    The 7x11 local window becomes a 1-D band |ds'| <= 83, so each 128-query
    tile only attends to 3 aligned 128-key chunks instead of 1024.
  - Exact per-chunk band is [128c-80, 128c+208): pairs at distance 81..83
    need |dj|=5 AND |di|>=1 simultaneously, which the 2-D mask forbids for
    the first/last 3 keys of a chunk, so halfwidth 80 suffices on the low
    side (and symmetrically 79+128 on the high side; 208 = 128+80).
  - Heads are padded to 64-partition slots (host-padded weights) so every
    engine access pattern starts at a 32-aligned partition.
  - All matmul operands are bf16; PSUM accumulation and softmax reductions
    stay fp32.
  - attnT[k, q] band tiles; exp on ScalarE (3 chunk-groups per head -> few,
    large activations); binary window mask applied multiplicatively, split
    between DVE and GPSIMD(Pool) for engine balance; @V uses
    lhsT=[V|0|ones|0] so softmax denominators land at partitions 64:112 of
    the same PSUM tile; the projection bias is folded in via a constant-1
    row of aoT.
  - po is a single persistent 2-bank PSUM tile; q-tile t of every head uses
    column (t%2)*512+(t//2)*128 so concurrently-open accumulation groups
    alternate banks.  Closed tiles keep their data across same-bank
    start=True (only has_written bits are cleared), so the per-head divides
    run in 2 halves while the next head's @V already accumulates.
  - PSUM budget: pat 2x[128,1024] (4 banks) + proj 2x[128,512] (2 banks)
    + po [128,1024] (2 banks) = 8 banks exactly.
  - Evacuation copies (Q/K/V/out PSUM->SBUF) are split between ScalarE and
    DVE by static schedule tables; output stores go through the SP queue so
    DMA setup does not stall the Activation sequencer.
"""

import sys

sys.path.insert(0, "/opt/trn_rl_repo")

import numpy as np

import concourse.bass as bass
from concourse import bacc
import concourse.mybir as mybir
import concourse.tile as tile
from concourse.bass_utils import run_bass_kernel_spmd

# ---------------------------------------------------------------- constants
B, S, C = 32, 1024, 384
H, D = 8, 48
HI, WI = 16, 64
N_CORES = 8
BL = B // N_CORES  # samples per core
SCALE = float(D) ** -0.5
F32 = mybir.dt.float32
BF16 = mybir.dt.bfloat16
PD = BF16  # precision of expT / m01 / vv

# s' = 16*j + i  <->  s = 64*i + j ;  PERM[s'] = s
_sp = np.arange(S)
PERM = (_sp % HI) * WI + (_sp // HI)

NQT = S // 128  # 8 query tiles (and key chunks)
WPADQ = 64 * H  # padded Q (and K) section width: 512
WQW = 2 * WPADQ + C  # 1408

# exact per-chunk bands: key-chunk c attends to queries [QLO[c], QHI[c])
QLO = [max(0, 128 * c - 80) for c in range(NQT)]
QHI = [min(S, 128 * c + 208) for c in range(NQT)]
WC = [QHI[c] - QLO[c] for c in range(NQT)]
OFFC = list(np.cumsum([0] + WC[:-1]))
BAND_W = sum(WC)  # 2144

# chunk groups per PSUM pat tile (each group width <= 1024 f32 = 2 banks)
CH_GROUPS = [(0, 1, 2), (3, 4, 5), (6, 7)]
GRP_BASE = [OFFC[g[0]] for g in CH_GROUPS]
GRP_W = [OFFC[g[-1]] + WC[g[-1]] - OFFC[g[0]] for g in CH_GROUPS]
assert max(GRP_W) <= 1024

# po column slot for q-tile t (identical for every head; open groups t,t+1
# always land in different PSUM banks)
PO_COL = [(t % 2) * 512 + (t // 2) * 128 for t in range(NQT)]

# ------------------------------------------------- engine schedule tables
# 'a' = ScalarE(Act) copy, 'd' = DVE tensor_copy
QK_EVAC = "aadaaadaaadaaada"  # 16 per sample (Q then K, pair-major)
V_EVAC = "aadaaada"  # 8 per sample
OUT_EVAC = "aadaaada"  # 8 per sample
# 'd' = DVE, 'p' = Pool(GPSIMD) for the 3 mask multiplies of each head.
# g1 (chunks 3-5) sits mid-head where its latency hides; g0/g2 gate the
# @V start/tail chains, so they stay on the fast DVE.
MASK_ENG = ["dpd"] * H
# divide granularity: q-tile ranges; later ranges close later, and finer
# tail pieces release the po columns the next head's early @V chunks need.
DIV_RANGES = [(0, 4), (4, 6), (6, 8)]

# ---------------------------------------------------------------- bass program
_CACHE = {}


def _build():
    if "nc" in _CACHE:
        return _CACHE["nc"]

    nc = bacc.Bacc(None, target_bir_lowering=False)
    xT_d = nc.declare_dram_parameter("xT", [BL, C, S], BF16, isOutput=False)
    wq_d = nc.declare_dram_parameter("wq_pad", [C, WQW], BF16, isOutput=False)
    wp_d = nc.declare_dram_parameter("wp_pad", [4, 128, C], BF16, isOutput=False)
    ones_d = nc.declare_dram_parameter("ones_row", [1, S], BF16, isOutput=False)
    m_d = nc.declare_dram_parameter("m01", [128, BAND_W], PD, isOutput=False)
    out_d = nc.declare_dram_parameter("out", [BL, S, C], F32, isOutput=True)

    with tile.TileContext(nc) as tc:
        with (
            tc.tile_pool(name="singles", bufs=1) as singles,
            tc.tile_pool(name="xt_pool", bufs=3) as xt_pool,
            tc.tile_pool(name="out_pool", bufs=4) as out_pool,
            tc.tile_pool(name="ps_proj", bufs=2, space="PSUM") as ps_proj,
            tc.tile_pool(name="ps_pat", bufs=2, space="PSUM") as ps_pat,
            tc.tile_pool(name="ps_po", bufs=1, space="PSUM") as ps_po,
        ):
            # ---- constants.  Q/K/V weight sections load as separate DMAs so
            # the first projection matmuls only wait for their own section.
            w_sb = singles.tile([128, 3, WQW], BF16)
            wq_v = wq_d.rearrange("(c p) w -> p c w", p=128)
            nc.scalar.dma_start(w_sb[:, :, 0:WPADQ], wq_v[:, :, 0:WPADQ])
            nc.scalar.dma_start(
                w_sb[:, :, WPADQ : 2 * WPADQ], wq_v[:, :, WPADQ : 2 * WPADQ]
            )
            nc.scalar.dma_start(w_sb[:, :, 2 * WPADQ :], wq_v[:, :, 2 * WPADQ :])
            wp_sb = singles.tile([128, 4, C], BF16)
            nc.gpsimd.dma_start(wp_sb[:, :, :], wp_d.rearrange("f p c -> p f c"))
            m_sb = singles.tile([128, BAND_W], PD)
            nc.gpsimd.dma_start(m_sb, m_d[:, :])

            # persistent attention-output PSUM tile (2 banks)
            po = ps_po.tile([128, S], F32, tag="po")

            # ---- per-sample tiles, double-buffered for cross-sample overlap
            qTs, kTs, vvs, aoTs, expTs, dens = [], [], [], [], [], []
            for i in range(2):
                qTs.append(singles.tile([128, 4, S], BF16, name=f"qT{i}"))
                kTs.append(singles.tile([128, 4, S], BF16, name=f"kT{i}"))
                vvs.append(singles.tile([128, NQT, H, 128], PD, name=f"vv{i}"))
                aoTs.append(singles.tile([128, 4, S], BF16, name=f"aoT{i}"))
                expTs.append(singles.tile([128, BAND_W], PD, name=f"expT{i}"))
                dens.append(singles.tile([48, 2, 512], F32, name=f"den{i}"))
            def setup_set(i):
                """One-time constant sections of buffer set i (Pool engine).
                Set 1 is deferred until after proj(0) is issued so sample-0
                mask multiplies are not queued behind 20us of memsets."""
                vv, aoT = vvs[i], aoTs[i]
                nc.gpsimd.memset(vv[:, :, :, D : D + 16], 0.0)
                nc.gpsimd.memset(vv[:, :, :, D + 16 : 112], 1.0)
                nc.gpsimd.memset(vv[:, :, :, 112:128], 0.0)
                # zero dead rows (48:64, 112:128); starts must be 32-aligned so
                # cover 32:64 / 96:128 — live rows are rewritten by the divides.
                nc.gpsimd.memset(aoT[32:64, :, :], 0.0)
                nc.gpsimd.memset(aoT[96:128, :, :], 0.0)
                # constant-1 row: proj picks up b_proj from wp_pad[0][48]
                nc.gpsimd.dma_start(aoT[48:49, 0, :], ones_d[:, :])

            def evac(engine, dst, src):
                if engine == "a":
                    nc.scalar.copy(dst, src)
                else:
                    nc.vector.tensor_copy(dst, src)

            def load_xt(b):
                xt = xt_pool.tile([128, 3, S], BF16)
                nc.sync.dma_start(
                    xt[:, :, :], xT_d[b].rearrange("(c p) s -> p c s", p=128)
                )
                return xt

            def proj(b, xt):
                """QKV projection for sample b into qT/kT/vv set b%2."""
                qT, kT, vv = qTs[b % 2], kTs[b % 2], vvs[b % 2]
                ei = iter(QK_EVAC)
                for qk in range(2):
                    dst = qT if qk == 0 else kT
                    for pair in range(4):
                        ncol = qk * WPADQ + pair * 128
                        for half in range(2):
                            ps = ps_proj.tile([128, 512], F32, tag="mm")
                            for ci in range(3):
                                nc.tensor.matmul(
                                    ps[:, :],
                                    w_sb[:, ci, ncol : ncol + 128],
                                    xt[:, ci, half * 512 : (half + 1) * 512],
                                    start=(ci == 0),
                                    stop=(ci == 2),
                                )
                            seg = dst[:, pair, half * 512 : (half + 1) * 512]
                            evac(next(ei), seg, ps[:, :])
                vi = iter(V_EVAC)
                for st in range(NQT):
                    psv = ps_proj.tile([128, 512], F32, tag="mm")
                    for ci in range(3):
                        nc.tensor.matmul(
                            psv[:, 0:C],
                            xt[:, ci, st * 128 : (st + 1) * 128],
                            w_sb[:, ci, 2 * WPADQ : 2 * WPADQ + C],
                            start=(ci == 0),
                            stop=(ci == 2),
                        )
                    evac(
                        next(vi),
                        vv[:, st, :, 0:D],
                        psv[:, 0:C].rearrange("p (h d) -> p h d", h=H),
                    )

            def attention(b):
                qT, kT, vv, aoT = qTs[b % 2], kTs[b % 2], vvs[b % 2], aoTs[b % 2]
                for h in range(H):
                    pair, sub = divmod(h, 2)
                    p0 = sub * 64
                    expT = expTs[h % 2]
                    den_sb = dens[h % 2]
                    # ---- QK^T band, exp, mask (3 chunk-groups)
                    for gi, grp in enumerate(CH_GROUPS):
                        gbase, gw = GRP_BASE[gi], GRP_W[gi]
                        pat = ps_pat.tile([128, 1024], F32, tag="attn")
                        for c in grp:
                            lo = OFFC[c] - gbase
                            hi = lo + WC[c]
                            a = lo
                            while a < hi:
                                b2 = min(hi, (a // 512 + 1) * 512)
                                nc.tensor.matmul(
                                    pat[:, a:b2],
                                    kT[p0 : p0 + D, pair, c * 128 : (c + 1) * 128],
                                    qT[
                                        p0 : p0 + D,
                                        pair,
                                        QLO[c] + (a - lo) : QLO[c] + (b2 - lo),
                                    ],
                                    start=True,
                                    stop=True,
                                )
                                a = b2
                        nc.scalar.activation(
                            expT[:, gbase : gbase + gw],
                            pat[:, 0:gw],
                            mybir.ActivationFunctionType.Exp,
                            scale=SCALE,
                        )
                        eng = MASK_ENG[h][gi]
                        tt = (
                            nc.vector.tensor_tensor
                            if eng == "d"
                            else nc.gpsimd.tensor_tensor
                        )
                        tt(
                            expT[:, gbase : gbase + gw],
                            expT[:, gbase : gbase + gw],
                            m_sb[:, gbase : gbase + gw],
                            mybir.AluOpType.mult,
                        )
                    # ---- @V with ones rows at 64:112 -> denominators
                    for c in range(NQT):
                        lhsT = vv[:, c, h, :]
                        for t in range(max(c - 1, 0), min(c + 2, NQT)):
                            pc = PO_COL[t]
                            qs = max(128 * t, QLO[c])
                            qe = min(128 * t + 128, QHI[c])
                            nc.tensor.matmul(
                                po[:, pc + (qs - 128 * t) : pc + (qe - 128 * t)],
                                lhsT,
                                expT[:, OFFC[c] + (qs - QLO[c]) : OFFC[c] + (qe - QLO[c])],
                                start=(c == max(t - 1, 0)),
                                stop=(c == min(t + 1, NQT - 1)),
                            )
                    # ---- normalize in q-tile ranges: later ranges close later,
                    # finer tail pieces release po columns for the next head.
                    # q-order view of po: q-tile t = 2*thi + tlo lives at
                    # col tlo*512 + thi*128, so iterate (thi, tlo, u).
                    po_q = po[:, :].rearrange(
                        "p (tlo thi u) -> p thi tlo u", tlo=2, thi=4, u=128
                    )
                    den_flat = den_sb[:, :, :].rearrange("p a b -> p (a b)")
                    for ta, tb in DIV_RANGES:
                        nt = (tb - ta) // 2
                        den_v = den_flat[:, ta * 128 : tb * 128].rearrange(
                            "p (thi tlo u) -> p thi tlo u", thi=nt, tlo=2, u=128
                        )
                        ao_v = aoT[
                            p0 : p0 + D, pair, ta * 128 : tb * 128
                        ].rearrange("p (thi tlo u) -> p thi tlo u", thi=nt, tlo=2, u=128)
                        nc.vector.reciprocal(
                            den_v, po_q[64 : 64 + D, ta // 2 : tb // 2]
                        )
                        nc.vector.tensor_tensor(
                            ao_v,
                            po_q[0:D, ta // 2 : tb // 2],
                            den_v,
                            mybir.AluOpType.mult,
                        )

            def out_proj(b):
                aoT = aoTs[b % 2]
                oi = iter(OUT_EVAC)
                for sp in range(NQT // 2):  # store pairs of q-tiles
                    ot = out_pool.tile([128, 2, C], F32)
                    for k in range(2):
                        st = 2 * sp + k
                        psp = ps_proj.tile([128, 512], F32, tag="mm")
                        for p in range(4):
                            nc.tensor.matmul(
                                psp[:, 0:C],
                                aoT[:, p, st * 128 : (st + 1) * 128],
                                wp_sb[:, p, :],
                                start=(p == 0),
                                stop=(p == 3),
                            )
                        evac(next(oi), ot[:, k, :], psp[:, 0:C])
                    nc.sync.dma_start(
                        out_d[b, 2 * sp * 128 : (2 * sp + 2) * 128, :].rearrange(
                            "(k p) c -> p k c", k=2
                        ),
                        ot[:, :, :],
                    )

            # ---------------- main pipeline
            setup_set(0)
            xts = {0: load_xt(0)}
            proj(0, xts[0])
            setup_set(1)
            for b in range(BL):
                if b + 1 < BL:
                    xts[b + 1] = load_xt(b + 1)  # prefetch during attention(b)
                attention(b)
                if b + 1 < BL:
                    proj(b + 1, xts[b + 1])
                out_proj(b)

    nc.finalize()
    _CACHE["nc"] = nc
    return nc


# ---------------------------------------------------------------- host wrapper
def _np_bf16(a):
    import ml_dtypes

    return np.asarray(a, dtype=ml_dtypes.bfloat16)


def _build_m01(mask):
    """[128, BAND_W] banded 0/1 mask in exact-band layout (rows = key within
    chunk c, cols = q in [QLO[c], QHI[c]))."""
    mp = np.asarray(mask)[np.ix_(PERM, PERM)]
    good = np.isfinite(mp) & (mp == 0.0)
    m01 = np.zeros((128, BAND_W), np.float32)
    covered = 0
    for c in range(NQT):
        blk = good[QLO[c] : QHI[c], c * 128 : (c + 1) * 128]  # [q, k]
        m01[:, OFFC[c] : OFFC[c] + WC[c]] = blk.T.astype(np.float32)
        covered += int(blk.sum())
    assert covered == int(good.sum()), "mask not covered by band layout"
    return m01


def _pad_wqkv(w_qkv):
    """[384, 1152] -> [384, 1408]: Q/K head h at cols h*64..h*64+48 (zero pad),
    V kept natural at cols 1024:1408."""
    out = np.zeros((C, WQW), np.float32)
    for sec in range(2):  # Q, K
        for h in range(H):
            out[:, sec * WPADQ + h * 64 : sec * WPADQ + h * 64 + D] = w_qkv[
                :, sec * C + h * D : sec * C + (h + 1) * D
            ]
    out[:, 2 * WPADQ :] = w_qkv[:, 2 * C :]
    return out


def _pad_wproj(w_proj, b_proj):
    """[384, 384] -> [4, 128, 384]: pair p rows 0:48 = head 2p, 64:112 = head 2p+1.
    Row 48 of pair 0 carries b_proj (matched by the constant-1 row in aoT)."""
    out = np.zeros((4, 128, C), np.float32)
    for p in range(4):
        out[p, 0:D] = w_proj[(2 * p) * D : (2 * p + 1) * D]
        out[p, 64 : 64 + D] = w_proj[(2 * p + 1) * D : (2 * p + 2) * D]
    out[0, D] = b_proj
    return out


def kernel(x, w_qkv, w_proj, b_proj, mask):
    x = np.asarray(x, np.float32)
    w_qkv = np.asarray(w_qkv, np.float32)
    w_proj = np.asarray(w_proj, np.float32)
    b_proj = np.asarray(b_proj, np.float32)

    nc = _build()

    xT = _np_bf16(np.ascontiguousarray(x[:, PERM, :].transpose(0, 2, 1)))  # [B, C, S']
    wq_pad = _np_bf16(_pad_wqkv(w_qkv))
    wp_pad = _np_bf16(_pad_wproj(w_proj, b_proj))
    ones_row = _np_bf16(np.ones((1, S), np.float32))
    m01 = _build_m01(mask)
    if PD == BF16:
        m01 = _np_bf16(m01)

    in_maps = [
        {
            "xT": xT[c * BL : (c + 1) * BL],
            "wq_pad": wq_pad,
            "wp_pad": wp_pad,
            "ones_row": ones_row,
            "m01": m01,
        }
        for c in range(N_CORES)
    ]
    res = run_bass_kernel_spmd(nc, in_maps, list(range(N_CORES)))
    out_p = np.concatenate([res.results[c]["out"] for c in range(N_CORES)], axis=0)
    out = np.empty_like(out_p)
    out[:, PERM, :] = out_p
    return out
